# revision 12
# baseline (speedup 1.0000x reference)
"""FBPINN forward kernel for Trainium2 (8 NeuronCores, Bass/Tile).

Problem: N=262144 points x in [0,1); S=32 overlapping subdomains, each with
its own MLP (1 -> 128 -> 128 -> 128 -> 128 -> 1, tanh). Cosine^2
partition-of-unity windows, normalized across subdomains; output is the
windowed sum of per-subdomain MLP outputs at each point.

Key structure exploited: subdomain s has support x in ((s-0.5)/S, (s+1.5)/S).
Each point lies in the support of at most TWO subdomains, and which two is a
function of its half-cell k = floor(2*S*x) in [0, 64): k=2j -> (j-1, j),
k=2j+1 -> (j, j+1). So instead of the dense S x N evaluation the reference
does, we bucket points by half-cell (host side), pad each bucket to a fixed
capacity, and evaluate each bucket under exactly its two active subdomain
MLPs on-device: a 16x FLOP reduction with bitwise-equivalent semantics
(dropped terms all have window weight exactly 0).

Sharding: core c owns half-cells 8c..8c+7 (a contiguous x-range). It
evaluates the 16 (bucket, subdomain) pairs touching those cells and combines
them with host-precomputed window weights; no cross-core communication.

Device layout: activations are [width=128 partitions, points free]. The
output layer is computed with h4 as the *stationary* matmul operand and
W_out as the moving operand, which directly yields outputs transposed into
[128 points, cols] layout so the window combine runs at full 128-lane DVE
width.
"""

import numpy as np

S = 32
WIDTH = 128
N_CORES = 8
HC = 2 * S          # 64 half-cells
CELLS_PER_CORE = HC // N_CORES   # 8
C = 4608            # per-bucket padded capacity (mean 4096, sigma ~64)
NT = C // 128       # 36 columns in transposed point tiles
CHUNK = 512         # matmul moving-dim tile (fp32 PSUM bank)
GROUP = 1536        # ACT batch (3 PSUM banks)
NSLOT = 16
DEPTH_HID = 3
TOL = 1e-8

# slot -> (s_rel, k_rel): subdomain 4c+s_rel evaluated on owned cell 8c+k_rel
SLOTS = [(-1, 0), (0, 0), (0, 1), (0, 2), (1, 1), (1, 2), (1, 3), (1, 4),
         (2, 3), (2, 4), (2, 5), (2, 6), (3, 5), (3, 6), (3, 7), (4, 7)]
# owned bucket k_rel -> (slot of left subdomain, slot of right subdomain)
BUCKET_SLOTS = [(0, 1), (2, 4), (3, 5), (6, 8), (7, 9), (10, 12), (11, 13),
                (14, 15)]

_prog_cache = {}


def _split_waits(nc, mybir, max_waits=1):
    """walrus in this env rejects >1 embedded sem-wait per instruction
    (CTRL setupSyncWait limit). Hoist extras onto NoOps on the same engine
    immediately before the instruction (same engine program order =>
    identical sync semantics)."""
    for fn in nc.m.functions:
        for blk in fn.blocks:
            out = []
            for inst in blk.instructions:
                si = inst.sync_info
                waits = list(si.on_wait) if si is not None else []
                if len(waits) > max_waits:
                    keep = waits[-max_waits:]
                    for k, w in enumerate(waits[:-max_waits]):
                        out.append(mybir.InstNoOp(
                            name=f"{inst.name}-wsplit{k}", opcode="NoOp",
                            engine=inst.engine,
                            sync_info=mybir.SyncInfo(on_wait=[w], on_update=[]),
                            ins=[], outs=[]))
                    inst.sync_info = mybir.SyncInfo(
                        on_wait=keep, on_update=list(si.on_update))
                out.append(inst)
            blk.instructions[:] = out


def build_program(reps=1):
    """Build the SPMD Bass program (identical on all 8 cores)."""
    import concourse.bass as bass
    import concourse.tile as tile
    from concourse import mybir

    f32 = mybir.dt.float32
    f32r = mybir.dt.float32r
    Tanh = mybir.ActivationFunctionType.Tanh
    mult = mybir.AluOpType.mult

    nc = bass.Bass()
    ub_d = nc.declare_dram_parameter("ub", [NSLOT, C], f32r, isOutput=False)
    win_d = nc.declare_dram_parameter("win", [NSLOT, 1, WIDTH], f32r, isOutput=False)
    bin_d = nc.declare_dram_parameter("bin", [NSLOT, WIDTH, 1], f32, isOutput=False)
    whid_d = nc.declare_dram_parameter(
        "whid", [NSLOT, DEPTH_HID, WIDTH, WIDTH], f32r, isOutput=False)
    bhid_d = nc.declare_dram_parameter(
        "bhid", [NSLOT, WIDTH, DEPTH_HID], f32, isOutput=False)
    wout_d = nc.declare_dram_parameter("wout", [NSLOT, WIDTH, 2], f32r, isOutput=False)
    wl_d = nc.declare_dram_parameter("wl", [CELLS_PER_CORE, 128, NT], f32, isOutput=False)
    wr_d = nc.declare_dram_parameter("wr", [CELLS_PER_CORE, 128, NT], f32, isOutput=False)
    hb_d = nc.declare_dram_parameter("hb", [CELLS_PER_CORE, 128, NT], f32, isOutput=False)
    out_d = nc.declare_dram_parameter("out", [CELLS_PER_CORE, 128, NT], f32, isOutput=True)

    from contextlib import ExitStack, nullcontext

    with tile.TileContext(nc) as tc, ExitStack() as ctx:
        upool = ctx.enter_context(tc.tile_pool(name="upool", bufs=2))
        wpool = ctx.enter_context(tc.tile_pool(name="wpool", bufs=2))
        hpool = ctx.enter_context(tc.tile_pool(name="hpool", bufs=4))
        opool = ctx.enter_context(tc.tile_pool(name="opool", bufs=1))
        epool = ctx.enter_context(tc.tile_pool(name="epool", bufs=3))
        zpool = ctx.enter_context(tc.tile_pool(name="zpool", bufs=2, space="PSUM"))
        opsum = ctx.enter_context(tc.tile_pool(name="opsum", bufs=2, space="PSUM"))

        # reps>1 wraps the body in a HW loop — used only for benchmarking
        loop = (tc.For_i(0, reps, 1, hint_engines=(
            mybir.EngineType.PE, mybir.EngineType.Activation,
            mybir.EngineType.DVE, mybir.EngineType.SP))
            if reps > 1 else nullcontext())
        with loop:
            # all 16 slots' transposed outputs, kept resident
            outT = opool.tile([128, NSLOT * NT], f32)

            for j in range(NSLOT):
                u_sb = upool.tile([1, C], f32r, tag="u")
                nc.sync.dma_start(out=u_sb[:], in_=ub_d[j:j + 1, :])
                win_t = wpool.tile([1, WIDTH], f32r, tag="win")
                nc.sync.dma_start(out=win_t[:], in_=win_d[j])
                bin_t = wpool.tile([WIDTH, 1], f32, tag="bin")
                nc.sync.dma_start(out=bin_t[:], in_=bin_d[j])
                whid_t = wpool.tile([WIDTH, DEPTH_HID * WIDTH], f32r, tag="whid")
                for l in range(DEPTH_HID):
                    nc.sync.dma_start(
                        out=whid_t[:, l * WIDTH:(l + 1) * WIDTH], in_=whid_d[j, l])
                bhid_t = wpool.tile([WIDTH, DEPTH_HID], f32, tag="bhid")
                nc.sync.dma_start(out=bhid_t[:], in_=bhid_d[j])
                wout_t = wpool.tile([WIDTH, 2], f32r, tag="wout")
                nc.sync.dma_start(out=wout_t[:], in_=wout_d[j])

                # layer 1: z = W_in (x) u  (K=1 outer product), tanh via ACT
                h_prev = hpool.tile([128, C], f32r, tag="h")
                for g in range(0, C, GROUP):
                    gsz = min(GROUP, C - g)
                    zp = zpool.tile([128, GROUP], f32, tag="zp")
                    for c0 in range(0, gsz, CHUNK):
                        nc.tensor.matmul(
                            zp[:, c0:c0 + CHUNK],
                            lhsT=win_t[:],
                            rhs=u_sb[0:1, g + c0:g + c0 + CHUNK],
                            start=True, stop=True)
                    nc.scalar.activation(
                        h_prev[:, g:g + gsz], zp[:, 0:gsz], Tanh, bias=bin_t[:])

                # hidden layers
                for l in range(DEPTH_HID):
                    h_next = hpool.tile([128, C], f32r, tag="h")
                    for g in range(0, C, GROUP):
                        gsz = min(GROUP, C - g)
                        zp = zpool.tile([128, GROUP], f32, tag="zp")
                        for c0 in range(0, gsz, CHUNK):
                            nc.tensor.matmul(
                                zp[:, c0:c0 + CHUNK],
                                lhsT=whid_t[:, l * WIDTH:(l + 1) * WIDTH],
                                rhs=h_prev[:, g + c0:g + c0 + CHUNK],
                                start=True, stop=True)
                        nc.scalar.activation(
                            h_next[:, g:g + gsz], zp[:, 0:gsz], Tanh,
                            bias=bhid_t[:, l:l + 1])
                    h_prev = h_next

                # output layer, transposed: h4 stationary, W_out moving.
                # fp32r needs N>=2, so W_out is fed as two identical columns
                # and every other PSUM column is kept.
                op = opsum.tile([128, 2 * NT], f32, tag="op")
                for g in range(NT):
                    nc.tensor.matmul(
                        op[:, 2 * g:2 * g + 2],
                        lhsT=h_prev[:, g * 128:(g + 1) * 128],
                        rhs=wout_t[:],
                        start=True, stop=True)
                # strided DVE reads from PSUM crash the exec unit in context;
                # copy contiguously to SBUF first, take even columns there
                opc = epool.tile([128, 2 * NT], f32, tag="opc")
                nc.vector.tensor_copy(opc[:], op[:])
                opc_even = opc.rearrange("p (g two) -> p g two", two=2)[:, :, 0]
                nc.vector.tensor_copy(outT[:, j * NT:(j + 1) * NT], opc_even)

            # window combine per owned bucket
            for b in range(CELLS_PER_CORE):
                sl, sr = BUCKET_SLOTS[b]
                wl_t = epool.tile([128, NT], f32, tag="wl")
                nc.sync.dma_start(out=wl_t[:], in_=wl_d[b])
                wr_t = epool.tile([128, NT], f32, tag="wr")
                nc.sync.dma_start(out=wr_t[:], in_=wr_d[b])
                hb_t = epool.tile([128, NT], f32, tag="hb")
                nc.sync.dma_start(out=hb_t[:], in_=hb_d[b])
                acc = epool.tile([128, NT], f32, tag="acc")
                tmp = epool.tile([128, NT], f32, tag="tmp")
                nc.vector.tensor_tensor(
                    acc[:], outT[:, sl * NT:(sl + 1) * NT], wl_t[:], mult)
                nc.vector.tensor_tensor(
                    tmp[:], outT[:, sr * NT:(sr + 1) * NT], wr_t[:], mult)
                nc.vector.tensor_add(acc[:], acc[:], tmp[:])
                nc.vector.tensor_add(acc[:], acc[:], hb_t[:])
                nc.sync.dma_start(out=out_d[b], in_=acc[:])

    _split_waits(nc, mybir)
    return nc


def _window_raw(u):
    """cos^2(pi/2 u) windows with exact support cutoff, float64."""
    return np.where(np.abs(u) < 1.0, np.cos(0.5 * np.pi * u) ** 2, 0.0)


def prep_inputs(x, W_in, b_in, W_hid, b_hid, W_out, b_out, centers, scales):
    """Host-side bucketing/padding/window precompute. Returns (in_maps,
    scatter) where scatter = list over global cells of index arrays."""
    xf = np.asarray(x, np.float32).reshape(-1)
    n = xf.shape[0]
    cents = np.asarray(centers, np.float64).reshape(-1)
    scals = np.asarray(scales, np.float64).reshape(-1)
    bo = np.asarray(b_out, np.float64).reshape(-1)

    k_id = np.clip(np.floor(xf.astype(np.float64) * HC).astype(np.int64), 0, HC - 1)
    order = np.argsort(k_id, kind="stable")
    counts = np.bincount(k_id, minlength=HC)
    if counts.max() > C:
        return None, None  # caller falls back to dense path
    starts = np.zeros(HC + 1, np.int64)
    np.cumsum(counts, out=starts[1:])
    cell_idx = [order[starts[k]:starts[k + 1]] for k in range(HC)]

    in_maps = []
    for c in range(N_CORES):
        ub = np.zeros((NSLOT, C), np.float32)
        win = np.zeros((NSLOT, 1, WIDTH), np.float32)
        bin_ = np.zeros((NSLOT, WIDTH, 1), np.float32)
        whid = np.zeros((NSLOT, DEPTH_HID, WIDTH, WIDTH), np.float32)
        bhid = np.zeros((NSLOT, WIDTH, DEPTH_HID), np.float32)
        wout = np.zeros((NSLOT, WIDTH, 2), np.float32)
        for j, (s_rel, k_rel) in enumerate(SLOTS):
            s = 4 * c + s_rel
            k = CELLS_PER_CORE * c + k_rel
            if not (0 <= s < S):
                continue
            idx = cell_idx[k]
            xs = xf[idx].astype(np.float64)
            u = (xs - cents[s]) / scals[s]
            u_pad = ((k + 0.5) / HC - cents[s]) / scals[s]
            row = np.full(C, u_pad, np.float64)
            row[:len(idx)] = u
            ub[j] = row.astype(np.float32)
            win[j, 0, :] = np.asarray(W_in, np.float32)[s, :, 0]
            bin_[j, :, 0] = np.asarray(b_in, np.float32)[s]
            whid[j] = np.asarray(W_hid, np.float32)[s].transpose(0, 2, 1)
            bhid[j] = np.asarray(b_hid, np.float32)[s].T
            wout[j, :, 0] = np.asarray(W_out, np.float32)[s, 0, :]
            wout[j, :, 1] = wout[j, :, 0]

        wl = np.zeros((CELLS_PER_CORE, 128, NT), np.float32)
        wr = np.zeros((CELLS_PER_CORE, 128, NT), np.float32)
        hb = np.zeros((CELLS_PER_CORE, 128, NT), np.float32)
        for b in range(CELLS_PER_CORE):
            k = CELLS_PER_CORE * c + b
            j_cell = k // 2
            s_l, s_r = (j_cell - 1, j_cell) if k % 2 == 0 else (j_cell, j_cell + 1)
            idx = cell_idx[k]
            xs = xf[idx].astype(np.float64)
            raw_l = raw_r = 0.0
            if 0 <= s_l < S:
                raw_l = _window_raw((xs - cents[s_l]) / scals[s_l])
            if 0 <= s_r < S:
                raw_r = _window_raw((xs - cents[s_r]) / scals[s_r])
            denom = raw_l + raw_r + TOL
            wlv = np.zeros(C); wrv = np.zeros(C); hbv = np.zeros(C)
            if 0 <= s_l < S:
                wlv[:len(idx)] = raw_l / denom
                hbv[:len(idx)] += wlv[:len(idx)] * bo[s_l]
            if 0 <= s_r < S:
                wrv[:len(idx)] = raw_r / denom
                hbv[:len(idx)] += wrv[:len(idx)] * bo[s_r]
            wl[b] = wlv.astype(np.float32).reshape(NT, 128).T
            wr[b] = wrv.astype(np.float32).reshape(NT, 128).T
            hb[b] = hbv.astype(np.float32).reshape(NT, 128).T

        in_maps.append({"ub": ub, "win": win, "bin": bin_, "whid": whid,
                        "bhid": bhid, "wout": wout, "wl": wl, "wr": wr,
                        "hb": hb})
    return in_maps, (cell_idx, counts, n)


def unpack_outputs(results, scatter):
    cell_idx, counts, n = scatter
    total = np.zeros(n, np.float32)
    for k in range(HC):
        c, b = divmod(k, CELLS_PER_CORE)
        tilev = results[c]["out"]  # [CELLS_PER_CORE, 128, NT]
        vals = tilev[b].T.reshape(-1)
        total[cell_idx[k]] = vals[:counts[k]]
    return total


def _dense_fallback(x, W_in, b_in, W_hid, b_hid, W_out, b_out, centers, scales):
    """Numpy mirror of the reference; only for pathological (non-uniform)
    inputs whose bucket counts overflow the compiled capacity."""
    xf = np.asarray(x, np.float32)
    u = (xf[None, :, :] - np.asarray(centers, np.float32)[:, None, :]) \
        / np.asarray(scales, np.float32)[:, None, :]
    raw = np.prod(np.where(np.abs(u) < 1.0,
                           np.cos(0.5 * np.pi * u) ** 2, 0.0), axis=-1)
    w = raw / (np.sum(raw, axis=0, keepdims=True) + TOL)
    total = np.zeros(xf.shape[0], np.float32)
    for s in range(S):
        h = np.tanh(u[s] @ np.asarray(W_in, np.float32)[s].T
                    + np.asarray(b_in, np.float32)[s])
        for l in range(DEPTH_HID):
            h = np.tanh(h @ np.asarray(W_hid, np.float32)[s, l].T
                        + np.asarray(b_hid, np.float32)[s, l])
        out = h @ np.asarray(W_out, np.float32)[s].T + np.asarray(b_out, np.float32)[s]
        total = total + w[s] * out[:, 0]
    return total


def get_program(reps=1):
    key = ("nc", reps)
    if key not in _prog_cache:
        _prog_cache[key] = build_program(reps)
    return _prog_cache[key]


def kernel(x, W_in, b_in, W_hid, b_hid, W_out, b_out, centers, scales):
    in_maps, scatter = prep_inputs(x, W_in, b_in, W_hid, b_hid, W_out, b_out,
                                   centers, scales)
    if in_maps is None:
        return _dense_fallback(x, W_in, b_in, W_hid, b_hid, W_out, b_out,
                               centers, scales)
    from concourse.bass_utils import run_bass_kernel_spmd
    nc = get_program()
    res = run_bass_kernel_spmd(nc, in_maps, list(range(N_CORES)))
    return unpack_outputs(res.results, scatter)


# revision 16
# speedup vs baseline: 1.7187x; 1.7187x over previous
"""FBPINN forward kernel for Trainium2 (8 NeuronCores, Bass/Tile).

Problem: N=262144 points x in [0,1); S=32 overlapping subdomains, each with
its own MLP (1 -> 128 -> 128 -> 128 -> 128 -> 1, tanh). Cosine^2
partition-of-unity windows, normalized across subdomains; output is the
windowed sum of per-subdomain MLP outputs at each point.

Key structure exploited: subdomain s has support x in ((s-0.5)/S, (s+1.5)/S).
Each point lies in the support of at most TWO subdomains, and which two is a
function of its half-cell k = floor(2*S*x) in [0, 64): k=2j -> (j-1, j),
k=2j+1 -> (j, j+1). Instead of the dense S x N evaluation the reference
does, points are bucketed by half-cell (host side), each bucket padded to a
fixed capacity, and each bucket evaluated under exactly its two active
subdomain MLPs on-device: a 16x FLOP reduction with identical semantics
(every dropped term has window weight exactly 0).

Sharding: core c owns half-cells 8c..8c+7 (a contiguous x-range). It
evaluates the 16 (bucket, subdomain) pairs touching those cells; no
cross-core communication. Window weights are precomputed on host (O(N),
0.1% of the FLOPs) and applied during the gather/unshard step.

Device: activations live as [width=128 partitions, points free]; matmuls run
in float32r (full PE rate, ~tf32 precision); tanh+bias fuse into one ACT
instruction reading PSUM directly. The output layer keeps W_out stationary
(2 duplicated columns; fp32r needs moving-free >= 2) so each 512-point chunk
is one cheap matmul producing an output row.
"""

import numpy as np

S = 32
WIDTH = 128
N_CORES = 8
HC = 2 * S          # 64 half-cells
CELLS_PER_CORE = HC // N_CORES   # 8
C = 4352            # per-bucket padded capacity (uniform N: mean 4096, max ~4290)
CHUNK = 512         # matmul moving-dim tile (one fp32 PSUM bank)
GROUPS = (1536, 1536, 1280)      # ACT batches (PSUM bank groups), sum = C
NSLOT = 16
DEPTH_HID = 3
TOL = 1e-8
PKC = 518           # packed param cols per slot: 3*128 whid | bin | 3 bhid | 2 wout | 128 win

# slot -> (s_rel, k_rel): subdomain 4c+s_rel evaluated on owned cell 8c+k_rel
SLOTS = [(-1, 0), (0, 0), (0, 1), (0, 2), (1, 1), (1, 2), (1, 3), (1, 4),
         (2, 3), (2, 4), (2, 5), (2, 6), (3, 5), (3, 6), (3, 7), (4, 7)]
# owned bucket k_rel -> (slot of left subdomain, slot of right subdomain)
BUCKET_SLOTS = [(0, 1), (2, 4), (3, 5), (6, 8), (7, 9), (10, 12), (11, 13),
                (14, 15)]

_prog_cache = {}


def _split_waits(nc, mybir, max_waits=1):
    """walrus in this env rejects >1 embedded sem-wait per instruction
    (CTRL setupSyncWait limit). Hoist extras onto NoOps on the same engine
    immediately before the instruction (same engine program order =>
    identical sync semantics)."""
    for fn in nc.m.functions:
        for blk in fn.blocks:
            out = []
            for inst in blk.instructions:
                si = inst.sync_info
                waits = list(si.on_wait) if si is not None else []
                if len(waits) > max_waits:
                    keep = waits[-max_waits:]
                    for k, w in enumerate(waits[:-max_waits]):
                        out.append(mybir.InstNoOp(
                            name=f"{inst.name}-wsplit{k}", opcode="NoOp",
                            engine=inst.engine,
                            sync_info=mybir.SyncInfo(on_wait=[w], on_update=[]),
                            ins=[], outs=[]))
                    inst.sync_info = mybir.SyncInfo(
                        on_wait=keep, on_update=list(si.on_update))
                out.append(inst)
            blk.instructions[:] = out


def build_program(reps=1):
    """Build the SPMD Bass program (identical on all 8 cores)."""
    import concourse.bass as bass
    import concourse.tile as tile
    from concourse import mybir
    from contextlib import ExitStack, nullcontext

    f32 = mybir.dt.float32
    f32r = mybir.dt.float32r
    Tanh = mybir.ActivationFunctionType.Tanh

    nc = bass.Bass()
    ub_d = nc.declare_dram_parameter("ub", [NSLOT, C], f32r, isOutput=False)
    wpk_d = nc.declare_dram_parameter("wpk", [128, NSLOT * PKC], f32r, isOutput=False)
    orow_d = nc.declare_dram_parameter("orow", [NSLOT, C], f32, isOutput=True)

    with tile.TileContext(nc) as tc, ExitStack() as ctx:
        upool = ctx.enter_context(tc.tile_pool(name="upool", bufs=3))
        wpool = ctx.enter_context(tc.tile_pool(name="wpool", bufs=1))
        hpool = ctx.enter_context(tc.tile_pool(name="hpool", bufs=3))
        rpool = ctx.enter_context(tc.tile_pool(name="rpool", bufs=2))
        zpool = ctx.enter_context(tc.tile_pool(name="zpool", bufs=2, space="PSUM"))
        opsum = ctx.enter_context(tc.tile_pool(name="opsum", bufs=2, space="PSUM"))

        # reps>1 wraps the body in a HW loop — used only for benchmarking
        loop = (tc.For_i(0, reps, 1, hint_engines=(
            mybir.EngineType.PE, mybir.EngineType.Activation,
            mybir.EngineType.DVE, mybir.EngineType.SP))
            if reps > 1 else nullcontext())
        with loop:
            wpk = wpool.tile([128, NSLOT * PKC], f32r)
            nc.sync.dma_start(out=wpk[:], in_=wpk_d[:])

            for j in range(NSLOT):
                base = j * PKC
                whid = wpk[:, base:base + 384]
                bin_t = wpk[:, base + 384:base + 385].bitcast(f32)
                bhid = wpk[:, base + 385:base + 388].bitcast(f32)
                wout = wpk[:, base + 388:base + 390]
                win = wpk[0:1, base + 390:base + 518]

                u_sb = upool.tile([1, C], f32r, tag="u")
                nc.sync.dma_start(out=u_sb[:], in_=ub_d[j:j + 1, :])

                # layer 1: z = W_in (x) u  (K=1 outer product); tanh+bias ACT
                h_prev = hpool.tile([128, C], f32r, tag="h")
                g0 = 0
                for gsz in GROUPS:
                    zp = zpool.tile([128, GROUPS[0]], f32, tag="zp")
                    for c0 in range(0, gsz, CHUNK):
                        cs = min(CHUNK, gsz - c0)
                        nc.tensor.matmul(
                            zp[:, c0:c0 + cs],
                            lhsT=win,
                            rhs=u_sb[0:1, g0 + c0:g0 + c0 + cs],
                            start=True, stop=True)
                    nc.scalar.activation(
                        h_prev[:, g0:g0 + gsz], zp[:, 0:gsz], Tanh, bias=bin_t)
                    g0 += gsz

                # hidden layers
                for l in range(DEPTH_HID):
                    h_next = hpool.tile([128, C], f32r, tag="h")
                    g0 = 0
                    for gsz in GROUPS:
                        zp = zpool.tile([128, GROUPS[0]], f32, tag="zp")
                        for c0 in range(0, gsz, CHUNK):
                            cs = min(CHUNK, gsz - c0)
                            nc.tensor.matmul(
                                zp[:, c0:c0 + cs],
                                lhsT=whid[:, l * WIDTH:(l + 1) * WIDTH],
                                rhs=h_prev[:, g0 + c0:g0 + c0 + cs],
                                start=True, stop=True)
                        nc.scalar.activation(
                            h_next[:, g0:g0 + gsz], zp[:, 0:gsz], Tanh,
                            bias=bhid[:, l:l + 1])
                        g0 += gsz
                    h_prev = h_next

                # output layer: W_out stationary (M=2, duplicated), h4 moving.
                # each chunk makes one output row segment; DVE stages row 0
                # to SBUF (DMA cannot read PSUM), then one DMA out per slot.
                rows = rpool.tile([1, C], f32, tag="rows")
                for c0 in range(0, C, CHUNK):
                    cs = min(CHUNK, C - c0)
                    op = opsum.tile([2, CHUNK], f32, tag="op")
                    nc.tensor.matmul(
                        op[:, 0:cs],
                        lhsT=wout,
                        rhs=h_prev[:, c0:c0 + cs],
                        start=True, stop=True)
                    nc.vector.tensor_copy(rows[0:1, c0:c0 + cs], op[0:1, 0:cs])
                nc.sync.dma_start(out=orow_d[j:j + 1, :], in_=rows[:])

    _split_waits(nc, mybir)
    return nc


def _window_raw(u):
    """cos^2(pi/2 u) windows with exact support cutoff, float64."""
    return np.where(np.abs(u) < 1.0, np.cos(0.5 * np.pi * u) ** 2, 0.0)


def prep_inputs(x, W_in, b_in, W_hid, b_hid, W_out, b_out, centers, scales):
    """Host-side bucketing/padding/packing. Returns (in_maps, combine) where
    combine carries everything needed to assemble the final output from the
    per-slot device output rows."""
    xf = np.asarray(x, np.float32).reshape(-1)
    n = xf.shape[0]
    cents = np.asarray(centers, np.float64).reshape(-1)
    scals = np.asarray(scales, np.float64).reshape(-1)
    bo = np.asarray(b_out, np.float64).reshape(-1)
    W_in = np.asarray(W_in, np.float32)
    b_in = np.asarray(b_in, np.float32)
    W_hid = np.asarray(W_hid, np.float32)
    b_hid = np.asarray(b_hid, np.float32)
    W_out = np.asarray(W_out, np.float32)

    k_id = np.clip(np.floor(xf.astype(np.float64) * HC).astype(np.int64), 0, HC - 1)
    order = np.argsort(k_id, kind="stable")
    counts = np.bincount(k_id, minlength=HC)
    if counts.max() > C:
        return None, None  # caller falls back to dense path
    starts = np.zeros(HC + 1, np.int64)
    np.cumsum(counts, out=starts[1:])
    cell_idx = [order[starts[k]:starts[k + 1]] for k in range(HC)]

    in_maps = []
    wl_all, wr_all, hb_all = [], [], []
    for c in range(N_CORES):
        ub = np.zeros((NSLOT, C), np.float32)
        wpk = np.zeros((128, NSLOT * PKC), np.float32)
        for j, (s_rel, k_rel) in enumerate(SLOTS):
            s = 4 * c + s_rel
            k = CELLS_PER_CORE * c + k_rel
            if not (0 <= s < S):
                continue
            idx = cell_idx[k]
            xs = xf[idx].astype(np.float64)
            u = (xs - cents[s]) / scals[s]
            u_pad = ((k + 0.5) / HC - cents[s]) / scals[s]
            row = np.full(C, u_pad, np.float64)
            row[:len(idx)] = u
            ub[j] = row.astype(np.float32)
            base = j * PKC
            wpk[:, base:base + 384] = np.concatenate(
                [W_hid[s, l].T for l in range(DEPTH_HID)], axis=1)
            wpk[:, base + 384] = b_in[s]
            wpk[:, base + 385:base + 388] = b_hid[s].T
            wpk[:, base + 388] = W_out[s, 0, :]
            wpk[:, base + 389] = W_out[s, 0, :]
            wpk[0, base + 390:base + 518] = W_in[s, :, 0]

        # window weights for the host-side combine
        wl_core, wr_core, hb_core = [], [], []
        for b in range(CELLS_PER_CORE):
            k = CELLS_PER_CORE * c + b
            j_cell = k // 2
            s_l, s_r = (j_cell - 1, j_cell) if k % 2 == 0 else (j_cell, j_cell + 1)
            idx = cell_idx[k]
            xs = xf[idx].astype(np.float64)
            raw_l = _window_raw((xs - cents[s_l]) / scals[s_l]) if 0 <= s_l < S else 0.0
            raw_r = _window_raw((xs - cents[s_r]) / scals[s_r]) if 0 <= s_r < S else 0.0
            denom = raw_l + raw_r + TOL
            wl = raw_l / denom if 0 <= s_l < S else np.zeros(len(idx))
            wr = raw_r / denom if 0 <= s_r < S else np.zeros(len(idx))
            hb = wl * (bo[s_l] if 0 <= s_l < S else 0.0) \
                + wr * (bo[s_r] if 0 <= s_r < S else 0.0)
            wl_core.append(wl); wr_core.append(wr); hb_core.append(hb)
        wl_all.append(wl_core); wr_all.append(wr_core); hb_all.append(hb_core)

        in_maps.append({"ub": ub, "wpk": wpk})
    return in_maps, (cell_idx, counts, n, wl_all, wr_all, hb_all)


def unpack_outputs(results, combine):
    cell_idx, counts, n, wl_all, wr_all, hb_all = combine
    total = np.zeros(n, np.float64)
    for k in range(HC):
        c, b = divmod(k, CELLS_PER_CORE)
        sl, sr = BUCKET_SLOTS[b]
        cnt = counts[k]
        rows = results[c]["orow"]
        a = rows[sl][:cnt].astype(np.float64)
        bb = rows[sr][:cnt].astype(np.float64)
        total[cell_idx[k]] = (wl_all[c][b] * a + wr_all[c][b] * bb
                              + hb_all[c][b])
    return total.astype(np.float32)


def _dense_fallback(x, W_in, b_in, W_hid, b_hid, W_out, b_out, centers, scales):
    """Numpy mirror of the reference; only for pathological (non-uniform)
    inputs whose bucket counts overflow the compiled capacity."""
    xf = np.asarray(x, np.float32)
    u = (xf[None, :, :] - np.asarray(centers, np.float32)[:, None, :]) \
        / np.asarray(scales, np.float32)[:, None, :]
    raw = np.prod(np.where(np.abs(u) < 1.0,
                           np.cos(0.5 * np.pi * u) ** 2, 0.0), axis=-1)
    w = raw / (np.sum(raw, axis=0, keepdims=True) + TOL)
    total = np.zeros(xf.shape[0], np.float32)
    for s in range(S):
        h = np.tanh(u[s] @ np.asarray(W_in, np.float32)[s].T
                    + np.asarray(b_in, np.float32)[s])
        for l in range(DEPTH_HID):
            h = np.tanh(h @ np.asarray(W_hid, np.float32)[s, l].T
                        + np.asarray(b_hid, np.float32)[s, l])
        out = h @ np.asarray(W_out, np.float32)[s].T + np.asarray(b_out, np.float32)[s]
        total = total + w[s] * out[:, 0]
    return total


def get_program(reps=1):
    key = ("nc", reps)
    if key not in _prog_cache:
        _prog_cache[key] = build_program(reps)
    return _prog_cache[key]


def kernel(x, W_in, b_in, W_hid, b_hid, W_out, b_out, centers, scales):
    in_maps, combine = prep_inputs(x, W_in, b_in, W_hid, b_hid, W_out, b_out,
                                   centers, scales)
    if in_maps is None:
        return _dense_fallback(x, W_in, b_in, W_hid, b_hid, W_out, b_out,
                               centers, scales)
    from concourse.bass_utils import run_bass_kernel_spmd
    nc = get_program()
    res = run_bass_kernel_spmd(nc, in_maps, list(range(N_CORES)))
    return unpack_outputs(res.results, combine)


# revision 18
# speedup vs baseline: 1.7417x; 1.0134x over previous
"""FBPINN forward kernel for Trainium2 (8 NeuronCores, Bass/Tile).

Problem: N=262144 points x in [0,1); S=32 overlapping subdomains, each with
its own MLP (1 -> 128 -> 128 -> 128 -> 128 -> 1, tanh). Cosine^2
partition-of-unity windows, normalized across subdomains; output is the
windowed sum of per-subdomain MLP outputs at each point.

Key structure exploited: subdomain s has support x in ((s-0.5)/S, (s+1.5)/S).
Each point lies in the support of at most TWO subdomains, and which two is a
function of its half-cell k = floor(2*S*x) in [0, 64): k=2j -> (j-1, j),
k=2j+1 -> (j, j+1). Instead of the dense S x N evaluation the reference
does, points are bucketed by half-cell (host side), each bucket padded to a
fixed capacity, and each bucket evaluated under exactly its two active
subdomain MLPs on-device: a 16x FLOP reduction with identical semantics
(every dropped term has window weight exactly 0).

Sharding: core c owns half-cells 8c..8c+7 (a contiguous x-range). It
evaluates the 16 (bucket, subdomain) pairs touching those cells; no
cross-core communication. Window weights are precomputed on host (O(N),
0.1% of the FLOPs) and applied during the gather/unshard step.

Device: activations live as [width=128 partitions, points free]; matmuls run
in float32r (full PE rate, ~tf32 precision); tanh+bias fuse into one ACT
instruction reading PSUM directly. The output layer keeps W_out stationary
(2 duplicated columns; fp32r needs moving-free >= 2) so each 512-point chunk
is one cheap matmul producing an output row.
"""

import numpy as np

S = 32
WIDTH = 128
N_CORES = 8
HC = 2 * S          # 64 half-cells
CELLS_PER_CORE = HC // N_CORES   # 8
C = 4352            # per-bucket padded capacity (uniform N: mean 4096, max ~4290)
CHUNK = 512         # matmul moving-dim tile (one fp32 PSUM bank)
GROUPS = (1536, 1536, 1280)      # ACT batches (PSUM bank groups), sum = C
NSLOT = 16
DEPTH_HID = 3
TOL = 1e-8
PKC = 518           # packed param cols per slot: 3*128 whid | bin | 3 bhid | 2 wout | 128 win

# slot -> (s_rel, k_rel): subdomain 4c+s_rel evaluated on owned cell 8c+k_rel
SLOTS = [(-1, 0), (0, 0), (0, 1), (0, 2), (1, 1), (1, 2), (1, 3), (1, 4),
         (2, 3), (2, 4), (2, 5), (2, 6), (3, 5), (3, 6), (3, 7), (4, 7)]
# owned bucket k_rel -> (slot of left subdomain, slot of right subdomain)
BUCKET_SLOTS = [(0, 1), (2, 4), (3, 5), (6, 8), (7, 9), (10, 12), (11, 13),
                (14, 15)]

_prog_cache = {}


def _split_waits(nc, mybir, max_waits=1):
    """walrus in this env rejects >1 embedded sem-wait per instruction
    (CTRL setupSyncWait limit). Hoist extras onto NoOps on the same engine
    immediately before the instruction (same engine program order =>
    identical sync semantics)."""
    for fn in nc.m.functions:
        for blk in fn.blocks:
            out = []
            for inst in blk.instructions:
                si = inst.sync_info
                waits = list(si.on_wait) if si is not None else []
                if len(waits) > max_waits:
                    keep = waits[-max_waits:]
                    for k, w in enumerate(waits[:-max_waits]):
                        out.append(mybir.InstNoOp(
                            name=f"{inst.name}-wsplit{k}", opcode="NoOp",
                            engine=inst.engine,
                            sync_info=mybir.SyncInfo(on_wait=[w], on_update=[]),
                            ins=[], outs=[]))
                    inst.sync_info = mybir.SyncInfo(
                        on_wait=keep, on_update=list(si.on_update))
                out.append(inst)
            blk.instructions[:] = out


def build_program(reps=1):
    """Build the SPMD Bass program (identical on all 8 cores)."""
    import concourse.bass as bass
    import concourse.tile as tile
    from concourse import mybir
    from contextlib import ExitStack, nullcontext

    f32 = mybir.dt.float32
    f32r = mybir.dt.float32r
    Tanh = mybir.ActivationFunctionType.Tanh

    nc = bass.Bass()
    ub_d = nc.declare_dram_parameter("ub", [NSLOT, C], f32r, isOutput=False)
    wpk_d = nc.declare_dram_parameter("wpk", [128, NSLOT * PKC], f32r, isOutput=False)
    orow_d = nc.declare_dram_parameter("orow", [NSLOT, C], f32, isOutput=True)

    with tile.TileContext(nc) as tc, ExitStack() as ctx:
        upool = ctx.enter_context(tc.tile_pool(name="upool", bufs=2))
        wpool = ctx.enter_context(tc.tile_pool(name="wpool", bufs=1))
        hpool = ctx.enter_context(tc.tile_pool(name="hpool", bufs=4))
        rpool = ctx.enter_context(tc.tile_pool(name="rpool", bufs=2))
        zpool = ctx.enter_context(tc.tile_pool(name="zpool", bufs=2, space="PSUM"))
        opsum = ctx.enter_context(tc.tile_pool(name="opsum", bufs=2, space="PSUM"))

        # reps>1 wraps the body in a HW loop — used only for benchmarking
        loop = (tc.For_i(0, reps, 1, hint_engines=(
            mybir.EngineType.PE, mybir.EngineType.Activation,
            mybir.EngineType.DVE, mybir.EngineType.SP))
            if reps > 1 else nullcontext())
        with loop:
            wpk = wpool.tile([128, NSLOT * PKC], f32r)

            for j in range(NSLOT):
                base = j * PKC
                nc.sync.dma_start(out=wpk[:, base:base + PKC],
                                  in_=wpk_d[:, base:base + PKC])
                whid = wpk[:, base:base + 384]
                bin_t = wpk[:, base + 384:base + 385].bitcast(f32)
                bhid = wpk[:, base + 385:base + 388].bitcast(f32)
                wout = wpk[:, base + 388:base + 390]
                win = wpk[0:1, base + 390:base + 518]

                u_sb = upool.tile([1, C], f32r, tag="u")
                nc.sync.dma_start(out=u_sb[:], in_=ub_d[j:j + 1, :])

                # layer 1: z = W_in (x) u  (K=1 outer product); tanh+bias ACT
                h_prev = hpool.tile([128, C], f32r, tag="h")
                g0 = 0
                for gsz in GROUPS:
                    zp = zpool.tile([128, GROUPS[0]], f32, tag="zp")
                    for c0 in range(0, gsz, CHUNK):
                        cs = min(CHUNK, gsz - c0)
                        nc.tensor.matmul(
                            zp[:, c0:c0 + cs],
                            lhsT=win,
                            rhs=u_sb[0:1, g0 + c0:g0 + c0 + cs],
                            start=True, stop=True)
                    nc.scalar.activation(
                        h_prev[:, g0:g0 + gsz], zp[:, 0:gsz], Tanh, bias=bin_t)
                    g0 += gsz

                # hidden layers
                for l in range(DEPTH_HID):
                    h_next = hpool.tile([128, C], f32r, tag="h")
                    g0 = 0
                    for gsz in GROUPS:
                        zp = zpool.tile([128, GROUPS[0]], f32, tag="zp")
                        for c0 in range(0, gsz, CHUNK):
                            cs = min(CHUNK, gsz - c0)
                            nc.tensor.matmul(
                                zp[:, c0:c0 + cs],
                                lhsT=whid[:, l * WIDTH:(l + 1) * WIDTH],
                                rhs=h_prev[:, g0 + c0:g0 + c0 + cs],
                                start=True, stop=True)
                        nc.scalar.activation(
                            h_next[:, g0:g0 + gsz], zp[:, 0:gsz], Tanh,
                            bias=bhid[:, l:l + 1])
                        g0 += gsz
                    h_prev = h_next

                # output layer: W_out stationary (M=2, duplicated), h4 moving.
                # each chunk makes one output row segment; DVE stages row 0
                # to SBUF (DMA cannot read PSUM), then one DMA out per slot.
                rows = rpool.tile([1, C], f32, tag="rows")
                for c0 in range(0, C, CHUNK):
                    cs = min(CHUNK, C - c0)
                    op = opsum.tile([2, CHUNK], f32, tag="op")
                    nc.tensor.matmul(
                        op[:, 0:cs],
                        lhsT=wout,
                        rhs=h_prev[:, c0:c0 + cs],
                        start=True, stop=True)
                    nc.vector.tensor_copy(rows[0:1, c0:c0 + cs], op[0:1, 0:cs])
                nc.sync.dma_start(out=orow_d[j:j + 1, :], in_=rows[:])

    _split_waits(nc, mybir)
    return nc


def _window_raw(u):
    """cos^2(pi/2 u) windows with exact support cutoff, float64."""
    return np.where(np.abs(u) < 1.0, np.cos(0.5 * np.pi * u) ** 2, 0.0)


def prep_inputs(x, W_in, b_in, W_hid, b_hid, W_out, b_out, centers, scales):
    """Host-side bucketing/padding/packing. Returns (in_maps, combine) where
    combine carries everything needed to assemble the final output from the
    per-slot device output rows."""
    xf = np.asarray(x, np.float32).reshape(-1)
    n = xf.shape[0]
    cents = np.asarray(centers, np.float64).reshape(-1)
    scals = np.asarray(scales, np.float64).reshape(-1)
    bo = np.asarray(b_out, np.float64).reshape(-1)
    W_in = np.asarray(W_in, np.float32)
    b_in = np.asarray(b_in, np.float32)
    W_hid = np.asarray(W_hid, np.float32)
    b_hid = np.asarray(b_hid, np.float32)
    W_out = np.asarray(W_out, np.float32)

    k_id = np.clip(np.floor(xf.astype(np.float64) * HC).astype(np.int64), 0, HC - 1)
    order = np.argsort(k_id, kind="stable")
    counts = np.bincount(k_id, minlength=HC)
    if counts.max() > C:
        return None, None  # caller falls back to dense path
    starts = np.zeros(HC + 1, np.int64)
    np.cumsum(counts, out=starts[1:])
    cell_idx = [order[starts[k]:starts[k + 1]] for k in range(HC)]

    in_maps = []
    wl_all, wr_all, hb_all = [], [], []
    for c in range(N_CORES):
        ub = np.zeros((NSLOT, C), np.float32)
        wpk = np.zeros((128, NSLOT * PKC), np.float32)
        for j, (s_rel, k_rel) in enumerate(SLOTS):
            s = 4 * c + s_rel
            k = CELLS_PER_CORE * c + k_rel
            if not (0 <= s < S):
                continue
            idx = cell_idx[k]
            xs = xf[idx].astype(np.float64)
            u = (xs - cents[s]) / scals[s]
            u_pad = ((k + 0.5) / HC - cents[s]) / scals[s]
            row = np.full(C, u_pad, np.float64)
            row[:len(idx)] = u
            ub[j] = row.astype(np.float32)
            base = j * PKC
            wpk[:, base:base + 384] = np.concatenate(
                [W_hid[s, l].T for l in range(DEPTH_HID)], axis=1)
            wpk[:, base + 384] = b_in[s]
            wpk[:, base + 385:base + 388] = b_hid[s].T
            wpk[:, base + 388] = W_out[s, 0, :]
            wpk[:, base + 389] = W_out[s, 0, :]
            wpk[0, base + 390:base + 518] = W_in[s, :, 0]

        # window weights for the host-side combine
        wl_core, wr_core, hb_core = [], [], []
        for b in range(CELLS_PER_CORE):
            k = CELLS_PER_CORE * c + b
            j_cell = k // 2
            s_l, s_r = (j_cell - 1, j_cell) if k % 2 == 0 else (j_cell, j_cell + 1)
            idx = cell_idx[k]
            xs = xf[idx].astype(np.float64)
            raw_l = _window_raw((xs - cents[s_l]) / scals[s_l]) if 0 <= s_l < S else 0.0
            raw_r = _window_raw((xs - cents[s_r]) / scals[s_r]) if 0 <= s_r < S else 0.0
            denom = raw_l + raw_r + TOL
            wl = raw_l / denom if 0 <= s_l < S else np.zeros(len(idx))
            wr = raw_r / denom if 0 <= s_r < S else np.zeros(len(idx))
            hb = wl * (bo[s_l] if 0 <= s_l < S else 0.0) \
                + wr * (bo[s_r] if 0 <= s_r < S else 0.0)
            wl_core.append(wl); wr_core.append(wr); hb_core.append(hb)
        wl_all.append(wl_core); wr_all.append(wr_core); hb_all.append(hb_core)

        in_maps.append({"ub": ub, "wpk": wpk})
    return in_maps, (cell_idx, counts, n, wl_all, wr_all, hb_all)


def unpack_outputs(results, combine):
    cell_idx, counts, n, wl_all, wr_all, hb_all = combine
    total = np.zeros(n, np.float64)
    for k in range(HC):
        c, b = divmod(k, CELLS_PER_CORE)
        sl, sr = BUCKET_SLOTS[b]
        cnt = counts[k]
        rows = results[c]["orow"]
        a = rows[sl][:cnt].astype(np.float64)
        bb = rows[sr][:cnt].astype(np.float64)
        total[cell_idx[k]] = (wl_all[c][b] * a + wr_all[c][b] * bb
                              + hb_all[c][b])
    return total.astype(np.float32)


def _dense_fallback(x, W_in, b_in, W_hid, b_hid, W_out, b_out, centers, scales):
    """Numpy mirror of the reference; only for pathological (non-uniform)
    inputs whose bucket counts overflow the compiled capacity."""
    xf = np.asarray(x, np.float32)
    u = (xf[None, :, :] - np.asarray(centers, np.float32)[:, None, :]) \
        / np.asarray(scales, np.float32)[:, None, :]
    raw = np.prod(np.where(np.abs(u) < 1.0,
                           np.cos(0.5 * np.pi * u) ** 2, 0.0), axis=-1)
    w = raw / (np.sum(raw, axis=0, keepdims=True) + TOL)
    total = np.zeros(xf.shape[0], np.float32)
    for s in range(S):
        h = np.tanh(u[s] @ np.asarray(W_in, np.float32)[s].T
                    + np.asarray(b_in, np.float32)[s])
        for l in range(DEPTH_HID):
            h = np.tanh(h @ np.asarray(W_hid, np.float32)[s, l].T
                        + np.asarray(b_hid, np.float32)[s, l])
        out = h @ np.asarray(W_out, np.float32)[s].T + np.asarray(b_out, np.float32)[s]
        total = total + w[s] * out[:, 0]
    return total


def get_program(reps=1):
    key = ("nc", reps)
    if key not in _prog_cache:
        _prog_cache[key] = build_program(reps)
    return _prog_cache[key]


def kernel(x, W_in, b_in, W_hid, b_hid, W_out, b_out, centers, scales):
    in_maps, combine = prep_inputs(x, W_in, b_in, W_hid, b_hid, W_out, b_out,
                                   centers, scales)
    if in_maps is None:
        return _dense_fallback(x, W_in, b_in, W_hid, b_hid, W_out, b_out,
                               centers, scales)
    from concourse.bass_utils import run_bass_kernel_spmd
    nc = get_program()
    res = run_bass_kernel_spmd(nc, in_maps, list(range(N_CORES)))
    return unpack_outputs(res.results, combine)
